# revision 3
# baseline (speedup 1.0000x reference)
"""Trainium2 Bass kernel v2 for nn_Attention_local (dense transformer block).

Data-parallel over batch: 8 images -> 8 NeuronCores, no collectives.

Structure (per core, one image, x [192, 16384] f32 -> host-cast bf16):
- Phase A (per image-half, per 128-row chunk of the 576 qkv rows):
    1x1 conv via PE (bias via ones-row K-augmentation) -> PSUM -> Act evac
    into a zero-padded conv tile cv [128, 66x130] bf16.
    Depthwise 3x3: 9 taps split between PE (diag-matmul into per-fy PSUM,
    Act evac w/ dw bias), DVE (scalar_tensor_tensor) and Pool; accumulator
    `acc` is PHASE-MAJOR bf16 [cnt, 16*512]: col = ph*512 + h1*32 + w1,
    ph = fy*4+fx (d-order within a head: ph-major, c-minor).
    q/k: per-phase squared-norms (DVE tensor_tensor_reduce); after both
    halves: rn = 1/max(sqrt(n2),eps); transposes via PE matmul with
    rhs = diag(rn*T) (q) / diag(rn) (k) -> PSUM -> Act evac -> staging ->
    DMA to qt_d/kt_d [1024, 3072] (col = 768h + 48ph + i, head-major).
    v: acc spilled to v_d [3072, 1024] in the same d-order via DMA.
- Phase B (per head): qt/kt/v tiles loaded from DRAM (bf16), attention
    entirely in PE: pa = kt^T qt (exp'd by Act), po = et @ v + ones-col
    for softmax_1 denominator; o = po/(sum+1) -> o_d.
- Phase C: o_d regathered channel-major (oc), 1x1 proj via PE with bias
    via ones-row, f32 result -> y.
"""

import numpy as np

HEADS = 4
C = 192
CO = 576
HW = 128
NPIX = HW * HW
HP = 8192            # pixels per half
CVF = 66 * 130       # padded conv tile free size per half
EPS = 1e-12

# tap split: taps t=0..8, (ky,kx) = (t//3, t%3)
# per-chunk (pe_taps, dve_taps, pool_taps); pool taps are 2-op (no stt on Pool)
# per-chunk (pe_taps, dve_stt_taps, act_split_taps, pool_split_taps):
# split taps: Act/Pool writes w*shift(cv) into a contiguous per-fy scratch in
# acc order; DVE adds it with the packed-bf16 2x tensor_tensor mode.
import os as _os
_CFG = _os.environ.get("TAPCFG", "5202")
_a, _d, _sa, _sp = (int(c) for c in _CFG)
_row = (tuple(range(_a)), tuple(range(_a, _a + _d)),
        tuple(range(_a + _d, _a + _d + _sa)),
        tuple(range(_a + _d + _sa, _a + _d + _sa + _sp)))
assert _a + _d + _sa + _sp == 9
TAP_SPLIT = [_row] * 5

# chunks of the 576 conv output rows: (cnt, segs); seg = (tensor, sr0, scnt, tr0)
# tensor: 0=q, 1=k, 2=v; sr0 = row within chunk; tr0 = row within tensor
CHUNKS = [
    (128, [(0, 0, 128, 0)]),
    (128, [(0, 0, 64, 128), (1, 64, 64, 0)]),
    (128, [(1, 0, 128, 64)]),
    (128, [(2, 0, 128, 0)]),
    (64, [(2, 0, 64, 128)]),
]

_COMPILED = {}


def _pieces(tr0, scnt):
    """Split tensor-local row range [tr0, tr0+scnt) at 48 (head) boundaries.
    Yields (h, i0, tl0, length): head, row-within-head, offset-within-seg."""
    out = []
    tr = tr0
    while tr < tr0 + scnt:
        h = tr // 48
        i0 = tr % 48
        ln = min(48 - i0, tr0 + scnt - tr)
        out.append((h, i0, tr - tr0, ln))
        tr += ln
    return out


def _build(debug_phase=3):
    import concourse.bass as bass
    import concourse.bacc as bacc
    import concourse.mybir as mybir
    from concourse.tile import TileContext
    from contextlib import ExitStack

    F32 = mybir.dt.float32
    BF16 = mybir.dt.bfloat16
    AF = mybir.ActivationFunctionType
    ALU = mybir.AluOpType

    nc = bacc.Bacc("TRN2", target_bir_lowering=False, debug=False)

    xa_d = nc.dram_tensor("xa", [128, NPIX], BF16, kind="ExternalInput")
    xb_d = nc.dram_tensor("xb", [65, NPIX], BF16, kind="ExternalInput")
    w1a_d = nc.dram_tensor("w1a", [128, CO], BF16, kind="ExternalInput")
    w1b_d = nc.dram_tensor("w1b", [65, CO], BF16, kind="ExternalInput")
    w16_d = nc.dram_tensor("w16", [CO, 16], F32, kind="ExternalInput")
    dg_d = nc.dram_tensor("dg", [45 * 128, 128], BF16, kind="ExternalInput")
    idb_d = nc.dram_tensor("idb", [128, 128], BF16, kind="ExternalInput")
    tpc_d = nc.dram_tensor("tpc", [C, 1], F32, kind="ExternalInput")
    pta_d = nc.dram_tensor("pta", [128, C], BF16, kind="ExternalInput")
    ptb_d = nc.dram_tensor("ptb", [65, C], BF16, kind="ExternalInput")
    on8_d = nc.dram_tensor("on8", [128, 8], BF16, kind="ExternalInput")
    onr_d = nc.dram_tensor("onr", [1, NPIX], BF16, kind="ExternalInput")
    y_d = nc.dram_tensor("y", [C, NPIX], BF16, kind="ExternalOutput")

    with TileContext(nc) as tc:
        with ExitStack() as es0:
            dram = es0.enter_context(tc.tile_pool(name="dram", bufs=1, space="DRAM"))
            qt_d = dram.tile([1024, 3072], BF16, tag="qt_d")
            kt_d = dram.tile([1024, 3072], BF16, tag="kt_d")
            v_d = dram.tile([3072, 1024], BF16, tag="v_d")
            o_d = dram.tile([3072, 1024], BF16, tag="o_d")
            row_d = dram.tile([8, 768], BF16, tag="row_d")

            const = es0.enter_context(tc.tile_pool(name="const", bufs=1))
            w1a = const.tile([128, CO], BF16, tag="w1a")
            w1b = const.tile([65, CO], BF16, tag="w1b")
            idb = const.tile([128, 128], BF16, tag="idb")
            tpc = const.tile([128, 2], F32, tag="tpc")
            on8 = const.tile([128, 8], BF16, tag="on8")
            fone = const.tile([128, 1], F32, tag="fone")
            nc.sync.dma_start(w1a[:], w1a_d.ap())
            nc.sync.dma_start(w1b[:], w1b_d.ap())
            nc.sync.dma_start(idb[:], idb_d.ap())
            nc.sync.dma_start(tpc[0:128, 0:1], tpc_d.ap()[0:128, :])
            nc.sync.dma_start(tpc[0:64, 1:2], tpc_d.ap()[128:192, :])
            nc.sync.dma_start(on8[:], on8_d.ap())
            nc.vector.memset(fone[:], 1.0)

            w16p = es0.enter_context(tc.tile_pool(name="w16p", bufs=1))
            w16s = []
            for m, (cnt, _) in enumerate(CHUNKS):
                w16 = w16p.tile([128, 16], F32, tag=f"w16_{m}")
                nc.sync.dma_start(w16[0:cnt, :], w16_d.ap()[m * 128:m * 128 + cnt, :])
                w16s.append(w16)
            dgp = es0.enter_context(tc.tile_pool(name="dgp", bufs=1))
            dgs = {}
            for m, (cnt, _) in enumerate(CHUNKS):
                for t in TAP_SPLIT[m][0]:
                    dg = dgp.tile([128, 128], BF16, tag=f"dg{m}_{t}", name=f"dg{m}_{t}")
                    nc.sync.dma_start(
                        dg[0:cnt, :], dg_d.ap()[(m * 9 + t) * 128:(m * 9 + t) * 128 + cnt, :])
                    dgs[(m, t)] = dg

            n2p = es0.enter_context(tc.tile_pool(name="n2p", bufs=1))
            n2s = {}   # (m, hf) -> [128, 16] f32
            for m in (0, 1, 2):
                for hf in (0, 1):
                    n2s[(m, hf)] = n2p.tile([128, 16], F32, tag=f"n2_{m}_{hf}", name=f"n2_{m}_{hf}")
            rns = {m: n2p.tile([128, 16], F32, tag=f"rn_{m}", name=f"rn_{m}") for m in (0, 1, 2)}

            # ---------------- phase A ----------------
            with ExitStack() as es1:
                xp = es1.enter_context(tc.tile_pool(name="xp", bufs=2))
                cvp = es1.enter_context(tc.tile_pool(name="cvp", bufs=2))
                accp = es1.enter_context(tc.tile_pool(name="accp", bufs=int(_os.environ.get("ACCB", "2"))))
                vap = es1.enter_context(tc.tile_pool(name="vap", bufs=2))
                p1p = es1.enter_context(tc.tile_pool(name="p1p", bufs=2, space="PSUM"))
                ptp = es1.enter_context(tc.tile_pool(name="ptp", bufs=1, space="PSUM"))
                tpp = es1.enter_context(tc.tile_pool(name="tpp", bufs=2, space="PSUM"))
                scp = es1.enter_context(tc.tile_pool(name="scp", bufs=2))
                stp = es1.enter_context(tc.tile_pool(name="stp", bufs=2))
                pscp = es1.enter_context(tc.tile_pool(name="pscp", bufs=3))

                accs = {}  # (m, hf) -> acc tile (q/k persist both halves; v spilled)

                def do_transposes(m, hf, acc):
                    """Unscaled transposes for q/k chunk m, one half (norm
                    scales are applied in phase B)."""
                    cnt, segs = CHUNKS[m]
                    for (tensor, sr0, scnt, tr0) in segs:
                        tgt = qt_d if tensor == 0 else kt_d
                        tgv = tgt[:].rearrange("n (h ph i) -> n h ph i", h=4, ph=16)
                        for j in range(4):
                            nck = hf * 4 + j
                            st = stp.tile([128, 2048], BF16, tag="st")
                            for pg in range(4):
                                tp = tpp.tile([128, 512], F32, tag="tp")
                                for pp in range(4):
                                    ph = pg * 4 + pp
                                    nc.tensor.matmul(
                                        tp[:, pp * scnt:(pp + 1) * scnt],
                                        acc[sr0:sr0 + scnt,
                                            ph * 512 + j * 128:ph * 512 + j * 128 + 128],
                                        idb[sr0:sr0 + scnt, sr0:sr0 + scnt],
                                        start=True, stop=True)
                                nc.scalar.activation(
                                    st[:, pg * 4 * scnt:(pg + 1) * 4 * scnt],
                                    tp[:, 0:4 * scnt], AF.Identity)
                            stv = st[:, 0:16 * scnt].rearrange(
                                "n (ph i) -> n ph i", ph=16)
                            for (h, i0, tl0, ln) in _pieces(tr0, scnt):
                                nc.sync.dma_start(
                                    tgv[nck * 128:(nck + 1) * 128, h, :, i0:i0 + ln],
                                    stv[:, :, tl0:tl0 + ln])

                def finalize_rn(m):
                    cnt, segs = CHUNKS[m]
                    rn = rns[m]
                    nc.vector.tensor_add(rn[0:cnt, :], n2s[(m, 0)][0:cnt, :],
                                         n2s[(m, 1)][0:cnt, :])
                    nc.scalar.sqrt(rn[0:cnt, :], rn[0:cnt, :])
                    nc.vector.tensor_scalar_max(rn[0:cnt, :], rn[0:cnt, :], EPS)
                    nc.vector.reciprocal(rn[0:cnt, :], rn[0:cnt, :])
                    for (tensor, sr0, scnt, tr0) in segs:
                        if tensor == 0:   # fold temperature into q row scales
                            tslc = tpc[tr0:tr0 + scnt, 0:1] if tr0 < 128 \
                                else tpc[tr0 - 128:tr0 - 128 + scnt, 1:2]
                            nc.vector.tensor_scalar_mul(
                                rn[sr0:sr0 + scnt, :], rn[sr0:sr0 + scnt, :], tslc)

                xhs = []
                for hf in (0, 1):
                    px0 = 0 if hf == 0 else 8064
                    xa = xp.tile([128, 8320], BF16, tag="xa", name=f"xa{hf}")
                    xb = xp.tile([65, 8320], BF16, tag="xb", name=f"xb{hf}")
                    nc.sync.dma_start(xa[:], xa_d.ap()[:, px0:px0 + 8320])
                    nc.sync.dma_start(xb[:], xb_d.ap()[:, px0:px0 + 8320])
                    xhs.append((xa, xb))
                for hf in (0, 1):
                    xa, xb = xhs[hf]
                    for m in (int(c) for c in _os.environ.get("CORDER", "13024")):
                        cnt, segs = CHUNKS[m]
                        w16 = w16s[m]
                        cv = cvp.tile([128, CVF], BF16, tag="cv")
                        cvv = cv[:].rearrange("c (r w) -> c r w", r=66)
                        # zero borders: pad row (top for hf0 / bottom for hf1) + cols
                        if hf == 0:
                            nc.gpsimd.memset(cvv[:, 0:1, :], 0.0)
                        else:
                            nc.gpsimd.memset(cvv[:, 65:66, :], 0.0)
                        nc.gpsimd.memset(cvv[:, :, 0:1], 0.0)
                        nc.gpsimd.memset(cvv[:, :, 129:130], 0.0)
                        # ---- 1x1 conv: 65 rows (64 core + 1 halo) ----
                        r0 = 1 if hf == 0 else 0   # cv row of first evac'd row
                        for nb in range(9):
                            p1 = p1p.tile([128, 1024], F32, tag="p1")
                            blk = 1024 if nb < 8 else 128
                            for s0 in range(0, blk, 512):
                                sz = min(512, blk - s0)
                                off = nb * 1024 + s0
                                nc.tensor.matmul(
                                    p1[0:cnt, s0:s0 + sz],
                                    w1a[:, m * 128:m * 128 + cnt],
                                    xa[:, off:off + sz], start=True, stop=False)
                                nc.tensor.matmul(
                                    p1[0:cnt, s0:s0 + sz],
                                    w1b[:, m * 128:m * 128 + cnt],
                                    xb[:, off:off + sz], start=False, stop=True)
                            nrow = blk // 128
                            nc.scalar.activation(
                                cvv[0:cnt, r0 + nb * 8:r0 + nb * 8 + nrow, 1:129],
                                p1[0:cnt, 0:blk], AF.Identity)
                        # ---- depthwise: acc phase-major bf16 [cnt, 8192] ----
                        if m >= 3:
                            acc = vap.tile([128, HP], BF16, tag="va", name=f"va{m}_{hf}")
                        else:
                            acc = accp.tile([128, HP], BF16, tag="accq", name=f"accq{m}_{hf}")
                        accs[(m, hf)] = acc
                        # PE taps -> per-fy PSUM [128, 1024] x2 -> Act evac (+db)
                        for fy in range(4):
                            for sb in range(2):   # h1l 0..7 / 8..15
                                pt = ptp.tile([128, 1024], F32, tag="pt")
                                pe_taps = TAP_SPLIT[m][0]
                                for g in range(2):   # 4 h1l rows each
                                    h1l0 = sb * 8 + g * 4
                                    for ti, t in enumerate(pe_taps):
                                        ky, kx = t // 3, t % 3
                                        rhs = bass.AP(
                                            cv.tensor,
                                            (4 * h1l0 + fy + ky) * 130 + kx,
                                            [[CVF, cnt], [520, 4], [1, 128]])
                                        nc.tensor.matmul(
                                            pt[0:cnt, g * 512:(g + 1) * 512],
                                            dgs[(m, t)][0:cnt, 0:cnt], rhs,
                                            start=(ti == 0),
                                            stop=(ti == len(pe_taps) - 1))
                                # evac: psum [h1l 8][x 128] -> acc [ph-major]
                                src = bass.AP(pt.tensor, 0,
                                              [[1024, cnt], [128, 8], [4, 32], [1, 4]])
                                dst = bass.AP(acc.tensor,
                                              fy * 2048 + sb * 8 * 32,
                                              [[HP, cnt], [32, 8], [1, 32], [512, 4]])
                                nc.scalar.activation(dst, src, AF.Identity,
                                                     bias=w16[0:cnt, 9:10])
                        # per-fy: DVE stt taps, then split taps (Act/Pool
                        # produce scaled copies in acc order; DVE 2x-adds them)
                        for fy in range(4):
                            for t in TAP_SPLIT[m][1]:
                                ky, kx = t // 3, t % 3
                                src = bass.AP(cv.tensor, (fy + ky) * 130 + kx,
                                              [[CVF, cnt], [520, 16], [1, 128]])
                                dst = bass.AP(acc.tensor, fy * 2048,
                                              [[HP, cnt], [32, 16], [1, 32], [512, 4]])
                                nc.vector.scalar_tensor_tensor(
                                    dst, src, w16[0:cnt, t:t + 1], dst,
                                    op0=ALU.mult, op1=ALU.add)
                            for eng, taps in ((0, TAP_SPLIT[m][2]),
                                              (1, TAP_SPLIT[m][3])):
                                for t in taps:
                                    ky, kx = t // 3, t % 3
                                    src = bass.AP(cv.tensor, (fy + ky) * 130 + kx,
                                                  [[CVF, cnt], [1, 4], [520, 16], [4, 32]])
                                    sc = pscp.tile([128, 2048], BF16, tag="psc")
                                    if eng == 0:
                                        nc.scalar.activation(
                                            sc[0:cnt, :], src, AF.Identity,
                                            scale=w16[0:cnt, t:t + 1])
                                    else:
                                        nc.gpsimd.tensor_scalar_mul(
                                            sc[0:cnt, :], src, w16[0:cnt, t:t + 1])
                                    dst = acc[0:cnt, fy * 2048:(fy + 1) * 2048]
                                    nc.vector.tensor_add(dst, dst, sc[0:cnt, :])
                        if m < 3:
                            # per-phase squared norms
                            n2h = n2s[(m, hf)]
                            use_act = _os.environ.get("NORMSPLIT", "0") == "1" and m < 2
                            for ph in range(16):
                                scr = scp.tile([128, 512], BF16, tag="scr")
                                if use_act:
                                    nc.scalar.activation(
                                        scr[0:cnt, :],
                                        acc[0:cnt, ph * 512:(ph + 1) * 512],
                                        AF.Square,
                                        accum_out=n2h[0:cnt, ph:ph + 1])
                                else:
                                    nc.vector.scalar_tensor_tensor(
                                        scr[0:cnt, :],
                                        acc[0:cnt, ph * 512:(ph + 1) * 512],
                                        fone[0:cnt, :],
                                        acc[0:cnt, ph * 512:(ph + 1) * 512],
                                        op0=ALU.mult, op1=ALU.mult,
                                        accum_out=n2h[0:cnt, ph:ph + 1])
                        else:
                            # spill v to v_d in head-major d-order
                            for (h, i0, tl0, ln) in _pieces(segs[0][3], segs[0][2]):
                                src = accs[(m, hf)][tl0:tl0 + ln, :].rearrange(
                                    "c (ph n) -> c ph n", ph=16)
                                dst = bass.AP(v_d.tensor,
                                              (768 * h + i0) * 1024 + hf * 512,
                                              [[1024, ln], [48 * 1024, 16], [1, 512]])
                                nc.sync.dma_start(dst, src)
                        if m < 3:
                            do_transposes(m, hf, acc)
                            if hf == 1:
                                finalize_rn(m)

            # ---------------- phase B: attention ----------------
            if debug_phase >= 2:
              with ExitStack() as es2:
                qtp = es2.enter_context(tc.tile_pool(name="qtp", bufs=16))
                ktp = es2.enter_context(tc.tile_pool(name="ktp", bufs=16))
                vxp = es2.enter_context(tc.tile_pool(name="vxp", bufs=12))
                etp = es2.enter_context(tc.tile_pool(name="etp", bufs=12))
                pap = es2.enter_context(tc.tile_pool(name="pap", bufs=int(_os.environ.get("PAPB", "2")), space="PSUM"))
                pop = es2.enter_context(tc.tile_pool(name="pop", bufs=int(_os.environ.get("POPB", "1")), space="PSUM"))
                zsp = es2.enter_context(tc.tile_pool(name="zsp", bufs=2, space="PSUM"))
                osp = es2.enter_context(tc.tile_pool(name="osp", bufs=3))
                zrp = es2.enter_context(tc.tile_pool(name="zrp", bufs=2))

                # map tensor-local row -> (chunk, row-in-chunk)
                def rn_src(tensor, tr):
                    if tensor == 0:
                        return (0, tr) if tr < 128 else (1, tr - 128)
                    return (1, 64 + tr) if tr < 64 else (2, tr - 64)

                scbp = es2.enter_context(tc.tile_pool(name="scbp", bufs=4))
                rowp = es2.enter_context(tc.tile_pool(name="rowp", bufs=4))
                for h in range(HEADS):
                    # gather q/k reciprocal-norm rows for this head -> [1, 768]
                    bcs = []
                    for tensor in (0, 1):
                        # row gathered i-major ([i,ph] flat); bcast permutes
                        row = rowp.tile([1, 768], F32, tag="row")
                        tr = 48 * h
                        while tr < 48 * (h + 1):
                            mm, r0 = rn_src(tensor, tr)
                            ln = min(48 * (h + 1) - tr, CHUNKS[mm][0] - r0)
                            i0 = tr - 48 * h
                            nc.sync.dma_start(row[0:1, i0 * 16:(i0 + ln) * 16],
                                              rns[mm][r0:r0 + ln, 0:16])
                            tr += ln
                        rowb = rowp.tile([1, 768], BF16, tag="rowb")
                        # cast + permute i-major -> ph-major on Act
                        nc.scalar.activation(
                            bass.AP(rowb.tensor, 0, [[768, 1], [48, 16], [1, 48]]),
                            bass.AP(row.tensor, 0, [[768, 1], [1, 16], [16, 48]]),
                            AF.Identity)
                        bc = scbp.tile([128, 768], BF16, tag="bc")
                        slot = h * 2 + tensor
                        nc.sync.dma_start(row_d[slot:slot + 1, :], rowb[:])
                        nc.sync.dma_start(
                            bc[:], bass.AP(row_d.tensor, slot * 768,
                                           [[0, 128], [1, 768]]))
                        bcs.append(bc)
                    qts, kts, vxs, ets = [], [], [], []
                    for nck in range(8):
                        qt = qtp.tile([128, 768], BF16, tag="qth")
                        nc.scalar.dma_start(
                            qt[:], qt_d[nck * 128:(nck + 1) * 128,
                                        768 * h:768 * (h + 1)])
                        nc.vector.tensor_mul(qt[:], qt[:], bcs[0][:])
                        qts.append(qt)
                        kt = ktp.tile([128, 768], BF16, tag="kth")
                        nc.scalar.dma_start(
                            kt[:], kt_d[nck * 128:(nck + 1) * 128,
                                        768 * h:768 * (h + 1)])
                        nc.vector.tensor_mul(kt[:], kt[:], bcs[1][:])
                        kts.append(kt)
                    for ec in range(6):
                        vx = vxp.tile([128, 1024], BF16, tag="vx")
                        nc.scalar.dma_start(
                            vx[:], v_d[768 * h + ec * 128:768 * h + (ec + 1) * 128, :])
                        vxs.append(vx)
                    for ec in range(6):
                        pa = pap.tile([128, 768], F32, tag="pa")
                        for s0 in (0, 512):
                            sz = 512 if s0 == 0 else 256
                            for nck in range(8):
                                nc.tensor.matmul(
                                    pa[:, s0:s0 + sz],
                                    kts[nck][:, ec * 128:(ec + 1) * 128],
                                    qts[nck][:, s0:s0 + sz],
                                    start=(nck == 0), stop=(nck == 7))
                        et = etp.tile([128, 768], BF16, tag="et")
                        nc.scalar.activation(et[:], pa[:], AF.Exp)
                        ets.append(et)
                    for dc in range(6):
                        po = pop.tile([128, 1024], F32, tag="po")
                        zs = zsp.tile([128, 8], F32, tag="zs")
                        for ec in range(6):
                            st_, sp_ = ec == 0, ec == 5
                            lhs = ets[ec][:, dc * 128:(dc + 1) * 128]
                            nc.tensor.matmul(po[:, 0:512], lhs, vxs[ec][:, 0:512],
                                             start=st_, stop=sp_)
                            nc.tensor.matmul(po[:, 512:1024], lhs, vxs[ec][:, 512:1024],
                                             start=st_, stop=sp_)
                            nc.tensor.matmul(zs[:], lhs, on8[:], start=st_, stop=sp_)
                        zr = zrp.tile([128, 1], F32, tag="zr")
                        nc.vector.tensor_scalar_add(zr[:], zs[:, 0:1], 1.0)
                        nc.vector.reciprocal(zr[:], zr[:])
                        ot = osp.tile([128, 1024], BF16, tag="ot")
                        nc.vector.tensor_scalar_mul(ot[:], po[:], zr[:])
                        nc.sync.dma_start(
                            o_d[768 * h + dc * 128:768 * h + (dc + 1) * 128, :], ot[:])

            # ---------------- phase C: projection ----------------
            if debug_phase >= 3:
              with ExitStack() as es3:
                pcp = es3.enter_context(tc.tile_pool(name="pcp", bufs=1))
                ocp = es3.enter_context(tc.tile_pool(name="ocp", bufs=1))
                ysp = es3.enter_context(tc.tile_pool(name="ysp", bufs=4))
                ppp = es3.enter_context(tc.tile_pool(name="ppp", bufs=4, space="PSUM"))
                pta = pcp.tile([128, C], BF16, tag="pta")
                ptb = pcp.tile([65, C], BF16, tag="ptb")
                nc.sync.dma_start(pta[:], pta_d.ap())
                nc.sync.dma_start(ptb[:], ptb_d.ap())
                oc1 = ocp.tile([128, NPIX], BF16, tag="oc1")
                oc2 = ocp.tile([65, NPIX], BF16, tag="oc2")
                nc.sync.dma_start(oc2[64:65, :], onr_d.ap())
                # gather o_d (row 768h+48ph+i) -> channel-major oc[c, ph*1024+n]
                for (dst, c0, ln0) in ((oc1, 0, 128), (oc2, 128, 64)):
                    for (h, i0, tl0, ln) in _pieces(c0, ln0):
                        src = bass.AP(o_d.tensor, (768 * h + i0) * 1024,
                                      [[1024, ln], [48 * 1024, 16], [1, 1024]])
                        d = dst[tl0:tl0 + ln, :].rearrange("c (ph n) -> c ph n", ph=16)
                        nc.sync.dma_start(d, src)
                yfulls = [ocp.tile([128, NPIX], BF16, tag="yf0", name="yf0"),
                          ocp.tile([128, NPIX], BF16, tag="yf1", name="yf1")]
                for mc, (m0, mcnt) in enumerate(((0, 128), (128, 64))):
                    yfull = yfulls[mc]
                    for nb in range(16):
                        # nb == phase ph: oc cols [ph*1024,(ph+1)*1024) = [h1, w1]
                        fy, fx = nb // 4, nb % 4
                        pp = ppp.tile([128, 1024], F32, tag="pp")
                        for s0 in (0, 512):
                            off = nb * 1024 + s0
                            nc.tensor.matmul(pp[0:mcnt, s0:s0 + 512],
                                             pta[:, m0:m0 + mcnt],
                                             oc1[:, off:off + 512],
                                             start=True, stop=False)
                            nc.tensor.matmul(pp[0:mcnt, s0:s0 + 512],
                                             ptb[:, m0:m0 + mcnt],
                                             oc2[:, off:off + 512],
                                             start=False, stop=True)
                        # scatter phase block to pixel positions in yfull
                        dst = bass.AP(yfull.tensor, fy * 128 + fx,
                                      [[NPIX, mcnt], [512, 32], [4, 32]])
                        nc.scalar.activation(dst, pp[0:mcnt, :], AF.Identity)
                    nc.sync.dma_start(y_d.ap()[m0:m0 + mcnt, :], yfull[0:mcnt, :])

            if debug_phase < 3:
                with ExitStack() as esd:
                    dpp = esd.enter_context(tc.tile_pool(name="dpp", bufs=1))
                    zt = dpp.tile([128, NPIX], F32, tag="zt")
                    nc.vector.memset(zt[:], 0.0)
                    nc.sync.dma_start(y_d.ap()[0:128, :], zt[:])
                    nc.sync.dma_start(y_d.ap()[128:192, :], zt[0:64, :])

    nc.compile()
    return nc


def kernel(**inputs):
    import ml_dtypes
    import concourse.bass_utils as bu

    BF = ml_dtypes.bfloat16

    x = np.asarray(inputs["x"], np.float32)
    qkv_w = np.asarray(inputs["qkv_w"], np.float32)
    qkv_b = np.asarray(inputs["qkv_b"], np.float32)
    dw_w = np.asarray(inputs["dw_w"], np.float32)
    dw_b = np.asarray(inputs["dw_b"], np.float32)
    proj_w = np.asarray(inputs["proj_w"], np.float32)
    proj_b = np.asarray(inputs["proj_b"], np.float32)
    temp = np.asarray(inputs["temperature"], np.float32).reshape(HEADS)

    if "nc" not in _COMPILED:
        _COMPILED["nc"] = _build()
    nc = _COMPILED["nc"]

    w1 = qkv_w.T  # [192, 576]
    w1b = np.concatenate([w1[128:192], qkv_b.reshape(1, CO)], axis=0)  # [65, 576]
    dw9 = dw_w.reshape(CO, 9)
    w16 = np.zeros((CO, 16), np.float32)
    w16[:, 0:9] = dw9
    w16[:, 9] = dw_b
    dg = np.zeros((45, 128, 128), np.float32)
    for m, (cnt, _) in enumerate(CHUNKS):
        for t in range(9):
            dg[m * 9 + t, :cnt, :cnt] = np.diag(dw9[m * 128:m * 128 + cnt, t])
    pt = proj_w.T  # [192, 192]
    ptb = np.concatenate([pt[128:192], proj_b.reshape(1, C)], axis=0)  # [65, 192]

    common = {
        "w1a": np.ascontiguousarray(w1[0:128]).astype(BF),
        "w1b": np.ascontiguousarray(w1b).astype(BF),
        "w16": np.ascontiguousarray(w16),
        "dg": np.ascontiguousarray(dg.reshape(45 * 128, 128)).astype(BF),
        "idb": np.eye(128, dtype=np.float32).astype(BF),
        "tpc": np.ascontiguousarray(np.repeat(temp, 48).reshape(C, 1)),
        "pta": np.ascontiguousarray(pt[0:128]).astype(BF),
        "ptb": np.ascontiguousarray(ptb).astype(BF),
        "on8": np.ones((128, 8), np.float32).astype(BF),
        "onr": np.ones((1, NPIX), np.float32).astype(BF),
    }
    in_maps = []
    for b in range(x.shape[0]):
        xf = x[b].reshape(C, NPIX)
        xaug = np.concatenate([xf[128:192], np.ones((1, NPIX), np.float32)], axis=0)
        in_maps.append({
            "xa": np.ascontiguousarray(xf[0:128]).astype(BF),
            "xb": np.ascontiguousarray(xaug).astype(BF),
            **common,
        })
    res = bu.run_bass_kernel_spmd(nc, in_maps, core_ids=list(range(len(in_maps))))
    out = np.stack([np.asarray(r["y"], np.float32).reshape(C, HW, HW)
                    for r in res.results])
    return out
